# revision 1
# baseline (speedup 1.0000x reference)
"""ConvLSTM attention pooling kernel for 8 Trainium2 NeuronCores.

Reference computation (per sample b):
    frames = x[b].reshape(chi, D)            # D = C*H*W = 65536, chi = 20
    scores = frames @ frames[-1] / chi       # [chi]
    alpha  = softmax(scores)                 # [chi]
    y      = x[b].reshape(D, chi) @ alpha    # [D]  (interleaved view!)

Sharding: pure data-parallel over batch B=64 -> 8 samples per core.

Two builders are kept:
  - _build_nc   : two-HBM-read version (one load per stage layout), ~266 us
                  per core on the cost model. Fallback (USE_Y = False).
  - _build_nc_y : single-HBM-read version (default). Loads x[b] once
                  contiguously, PE-transposes it on-chip into a column-major
                  layout that serves BOTH stages, computes the weighted sum
                  on the tensor engine via small per-sample "alpha scatter"
                  tiles, and writes each sample's output from one PSUM bank.
                  ~169 us/core on the cost model (two-read version: 266;
                  single-read DMA roofline: ~129). HW repeat-R marginal
                  measurements range 190-420 us across runs; the axon
                  dispatch floor (~60-100 ms, drifting) limits precision.
"""

import numpy as np

B = 64
CHI = 20
D = 64 * 32 * 32  # 65536
N_CORES = 8
S = B // N_CORES  # samples per core
P = 128
F = D // P  # 512

_CACHE = {}


def _build_nc(repeat=1):
    import concourse.bacc as bacc
    import concourse.tile as tile
    from concourse import mybir

    f32 = mybir.dt.float32
    nc = bacc.Bacc("TRN2", target_bir_lowering=False, debug=False)
    x_d = nc.dram_tensor("x", [S, CHI * D], f32, kind="ExternalInput").ap()
    y_d = nc.dram_tensor("y", [S, D], f32, kind="ExternalOutput").ap()

    with tile.TileContext(nc) as tc:
        with (
            tc.tile_pool(name="t1", bufs=2) as t1_pool,
            tc.tile_pool(name="t2", bufs=2) as t2_pool,
            tc.tile_pool(name="scratch", bufs=4) as s_pool,
            tc.tile_pool(name="small", bufs=4) as sm_pool,
            tc.tile_pool(name="outp", bufs=2) as o_pool,
            tc.tile_pool(name="singles", bufs=1) as ones_pool,
            tc.tile_pool(name="psum", bufs=2, space="PSUM") as p_pool,
        ):
            inv_chi_col = ones_pool.tile([P, 1], f32)
            nc.vector.memset(inv_chi_col, 1.0 / CHI)
            ones_row = ones_pool.tile([1, P], f32)
            nc.vector.memset(ones_row, 1.0)

            for _rep in range(repeat):
              for b in range(S):
                u = x_d[b]
                # frames layout: [p, c, f] <- u[c*D + p*F + f]
                t1 = t1_pool.tile([P, CHI, F], f32)
                nc.sync.dma_start(
                    out=t1, in_=u.rearrange("(c p f) -> p c f", p=P, f=F)
                )
                # interleaved layout: [p, f2, c] <- u[(p*F + f2)*CHI + c]
                t2 = t2_pool.tile([P, F, CHI], f32)
                nc.sync.dma_start(
                    out=t2, in_=u.rearrange("(p f c) -> p f c", p=P, c=CHI)
                )

                # ---- stage 1: scores ----
                partials = sm_pool.tile([P, CHI], f32)
                scratch = s_pool.tile([P, F], f32)
                for c in range(CHI):
                    # fused multiply + free-dim reduce: out = (in0 * 1) * in1,
                    # accum_out = sum(out) per partition
                    nc.vector.scalar_tensor_tensor(
                        out=scratch,
                        in0=t1[:, c, :],
                        scalar=1.0,
                        in1=t1[:, CHI - 1, :],
                        op0=mybir.AluOpType.mult,
                        op1=mybir.AluOpType.mult,
                        accum_out=partials[:, c : c + 1],
                    )

                s_psum = p_pool.tile([1, CHI], f32)
                nc.tensor.matmul(s_psum, inv_chi_col, partials, start=True, stop=True)
                scores = sm_pool.tile([1, CHI], f32)
                nc.vector.tensor_copy(out=scores, in_=s_psum)

                # ---- softmax on [1, CHI] ----
                neg_mx = sm_pool.tile([1, 1], f32)
                nc.vector.tensor_reduce(
                    out=neg_mx,
                    in_=scores,
                    axis=mybir.AxisListType.X,
                    op=mybir.AluOpType.max,
                    negate=True,
                )
                exps = sm_pool.tile([1, CHI], f32)
                sumexp = sm_pool.tile([1, 1], f32)
                nc.scalar.activation(
                    out=exps,
                    in_=scores,
                    func=mybir.ActivationFunctionType.Exp,
                    bias=neg_mx[:, 0:1],
                    scale=1.0,
                    accum_out=sumexp,
                )
                rsum = sm_pool.tile([1, 1], f32)
                nc.vector.reciprocal(rsum, sumexp)
                alpha = sm_pool.tile([1, CHI], f32)
                nc.vector.tensor_scalar_mul(alpha, exps, rsum)

                # broadcast alpha to all partitions: [128, CHI]
                b_psum = p_pool.tile([P, CHI], f32)
                nc.tensor.matmul(b_psum, ones_row, alpha, start=True, stop=True)
                alpha_bc = sm_pool.tile([P, CHI], f32)
                nc.scalar.copy(out=alpha_bc, in_=b_psum)

                # ---- stage 2: weighted sum over interleaved view ----
                out_t = o_pool.tile([P, F], f32)
                nc.vector.tensor_scalar_mul(out_t, t2[:, :, 0], alpha_bc[:, 0:1])
                for c in range(1, CHI):
                    nc.vector.scalar_tensor_tensor(
                        out=out_t,
                        in0=t2[:, :, c],
                        scalar=alpha_bc[:, c : c + 1],
                        in1=out_t,
                        op0=mybir.AluOpType.mult,
                        op1=mybir.AluOpType.add,
                    )

                nc.sync.dma_start(
                    out=y_d[b].rearrange("(p f) -> p f", p=P), in_=out_t
                )

    nc.compile()
    return nc


def _build_nc_y(ablate=(), repeat=1):
    """Single-HBM-read architecture.

    Per sample:
      1. load nt[128, 10240] = u (contiguous, one DMA)
      2. PE-transpose 128x128 blocks of nt -> "want" layout want[p', f] = u[128f + p']
         (psum banks of 4 blocks, copied to SBUF by DVE/ACT with strided dest)
      3. stage 1 (scores) on want: frame c = cols [512c, 512(c+1)) — 20 fused
         STT multiply+reduce ops, PE column-sum matmul, softmax -> alpha [1, 20]
      4. build 5 "alpha scatter" tiles rhs_s[128, 32]: rhs_s[p, g] =
         alpha[(128s+p) % 20] if g == (128s+p)//20 else 0  (zeroed + run-DMAs
         from an alpha column replicated twice)
      5. stage 2 on PE: windows mapped w = W + 16*m so psum partition m
         accumulates y[512m + 32W + g] — for W (16) and s (5):
         psum[:, W, :] += want[:, (5w+s) cols].T @ rhs_s
      6. ACT copy psum -> SBUF, then one DMA -> y[b] with 2KB-contiguous
         per-partition HBM runs (full DMA line rate)
    """
    import concourse.bacc as bacc
    import concourse.tile as tile
    from concourse import mybir

    f32 = mybir.dt.float32
    nc = bacc.Bacc("TRN2", target_bir_lowering=False, debug=False)
    x_d = nc.dram_tensor("x", [S, CHI * D], f32, kind="ExternalInput").ap()
    ident_d = nc.dram_tensor("ident", [P, P], f32, kind="ExternalInput").ap()
    ind1_d = nc.dram_tensor("ind1", [5, CHI, P], f32, kind="ExternalInput").ap()
    ind2_d = nc.dram_tensor("ind2", [5, P, 32], f32, kind="ExternalInput").ap()
    y_d = nc.dram_tensor("y", [S, D], f32, kind="ExternalOutput").ap()

    NW = 5  # columns per 640-element window (5 * 128)
    NG = 32  # groups (outputs) per window
    NB = 80  # 128-col blocks per sample (10240 / 128)
    NCHUNK = 16  # window chunks of 128 windows (2048 / 128)

    with tile.TileContext(nc) as tc:
        with (
            tc.tile_pool(name="nt", bufs=5) as nt_pool,
            tc.tile_pool(name="want", bufs=3) as want_pool,
            tc.tile_pool(name="scratch", bufs=4) as s_pool,
            tc.tile_pool(name="small", bufs=6) as sm_pool,
            tc.tile_pool(name="rhs", bufs=3) as rhs_pool,
            tc.tile_pool(name="osb", bufs=3) as osb_pool,
            tc.tile_pool(name="singles", bufs=1) as ones_pool,
            tc.tile_pool(name="pst", bufs=5, space="PSUM") as pst_pool,
            tc.tile_pool(name="pss", bufs=1, space="PSUM") as pss_pool,
            tc.tile_pool(name="pso", bufs=2, space="PSUM") as pso_pool,
        ):
            inv_chi_col = ones_pool.tile([P, 1], f32)
            nc.vector.memset(inv_chi_col, 1.0 / CHI)
            ident = ones_pool.tile([P, P], f32)
            nc.sync.dma_start(out=ident, in_=ident_d)
            ind1 = ones_pool.tile([CHI, 5, P], f32)
            nc.sync.dma_start(out=ind1, in_=ind1_d.rearrange("s c p -> c s p"))
            ind2 = ones_pool.tile([P, 5, 32], f32)
            nc.sync.dma_start(out=ind2, in_=ind2_d.rearrange("s p g -> p s g"))

            HB = NB // 4  # 20 blocks per quarter-load
            wants = [None] * S
            rhss = [None] * S

            def emit_load_transpose(b):
                """Load u[b] in halves and PE-transpose into want layout."""
                u = x_d[b].rearrange("(p q) -> p q", p=P)
                want = want_pool.tile([P, CHI * F], f32)
                wants[b] = want
                want_v = want.rearrange("p (pp j) -> p j pp", j=NB)
                for h in range(4):
                    nt = nt_pool.tile([P, HB * P], f32)
                    nc.sync.dma_start(out=nt, in_=u[:, h * HB * P : (h + 1) * HB * P])
                    for jg in range(HB // 4):  # 10 groups of 4 transposes
                        if "tc" in ablate:
                            break
                        ps = pst_pool.tile([P, 4, P], f32)
                        for dj in range(4):
                            j = jg * 4 + dj
                            nc.tensor.transpose(
                                ps[:, dj, :], nt[:, j * P : (j + 1) * P], ident
                            )
                        jga = h * (HB // 4) + jg
                        dst = want_v[:, jga * 4 : (jga + 1) * 4, :]
                        if jga % 10 == 0:  # ~1/10 of copies on DVE, rest on ACT
                            nc.vector.tensor_copy(out=dst, in_=ps)
                        else:
                            nc.scalar.copy(out=dst, in_=ps)

            partials_arr = [None] * S

            def emit_stage1_dve(b):
                """The DVE-heavy dot products for sample b (emitted early so
                the DVE stream is not head-of-line blocked behind copies)."""
                want = wants[b]
                wv = want.rearrange("p (c e) -> p c e", c=CHI)
                partials = sm_pool.tile([P, CHI], f32)
                scratch = s_pool.tile([P, F], f32)
                sq = s_pool.tile([P, F], f32)
                if "s1" in ablate:
                    nc.vector.memset(partials, 0.05)
                # c = 19: sum of squares on ACT (frees DVE)
                if "s1" not in ablate: nc.scalar.activation(
                    out=sq,
                    in_=wv[:, CHI - 1, :],
                    func=mybir.ActivationFunctionType.Square,
                    accum_out=partials[:, CHI - 1 : CHI],
                )
                for c in range(CHI - 1):
                    if "s1" in ablate: break
                    nc.vector.scalar_tensor_tensor(
                        out=scratch,
                        in0=wv[:, c, :],
                        scalar=1.0,
                        in1=wv[:, CHI - 1, :],
                        op0=mybir.AluOpType.mult,
                        op1=mybir.AluOpType.mult,
                        accum_out=partials[:, c : c + 1],
                    )

                partials_arr[b] = partials

            def emit_stage1_rest(b):
                """Scores matmul + softmax + alpha-scatter tiles."""
                partials = partials_arr[b]
                soft = pss_pool.tile([P, 48], f32)  # one psum bank, sliced
                s_psum = soft[0:1, 0:CHI]
                a_psum = soft[0:CHI, 24:25]
                a_pat = soft[:, 32:40]
                nc.tensor.matmul(s_psum, inv_chi_col, partials, start=True, stop=True)
                scores = sm_pool.tile([1, CHI], f32)
                nc.vector.tensor_copy(out=scores, in_=s_psum)

                neg_mx = sm_pool.tile([1, 1], f32)
                nc.vector.tensor_reduce(
                    out=neg_mx,
                    in_=scores,
                    axis=mybir.AxisListType.X,
                    op=mybir.AluOpType.max,
                    negate=True,
                )
                exps = sm_pool.tile([1, CHI], f32)
                sumexp = sm_pool.tile([1, 1], f32)
                nc.scalar.activation(
                    out=exps,
                    in_=scores,
                    func=mybir.ActivationFunctionType.Exp,
                    bias=neg_mx[:, 0:1],
                    scale=1.0,
                    accum_out=sumexp,
                )
                rsum = sm_pool.tile([1, 1], f32)
                nc.vector.reciprocal(rsum, sumexp)
                alpha = sm_pool.tile([1, CHI], f32)
                nc.vector.tensor_scalar_mul(alpha, exps, rsum)

                nc.tensor.transpose(a_psum, alpha, ident[:1, :1])
                a_one = sm_pool.tile([CHI, 1], f32)
                nc.scalar.copy(out=a_one, in_=a_psum)

                # a_pat[:, s] = ind1_s.T @ alpha_col = alpha[(128s+p) % 20]
                for s in range(NW):
                    nc.tensor.matmul(
                        a_pat[:, s : s + 1], ind1[:, s, :], a_one,
                        start=True, stop=True,
                    )
                # rhs_s[p, g] = ind2_s[p, g] * a_pat[p, s]
                rhs = rhs_pool.tile([P, NW, NG], f32)
                rhss[b] = rhs
                for s in range(NW):
                    nc.vector.tensor_scalar_mul(
                        rhs[:, s, :], ind2[:, s, :], a_pat[:, s : s + 1]
                    )

            def emit_stage2(b):
                """PE weighted sums + store for sample b."""
                want = wants[b]
                rhs = rhss[b]
                ob = pso_pool.tile([P, NCHUNK, NG], f32)
                want_w = want.rearrange("p (m s) -> p s m", s=NW)
                for W in range(NCHUNK):
                    if "s2" in ablate:
                        nc.tensor.matmul(ob[:, W, :], wants[b][:, :P], rhs[:, 0, :], start=True, stop=True)
                        continue
                    for s in range(NW):
                        nc.tensor.matmul(
                            ob[:, W, :],
                            want_w[:, s, W * P : (W + 1) * P],
                            rhs[:, s, :],
                            start=(s == 0),
                            stop=(s == NW - 1),
                        )
                out_sb = osb_pool.tile([P, NCHUNK, NG], f32)
                nc.scalar.copy(out=out_sb, in_=ob)
                nc.sync.dma_start(
                    out=y_d[b].rearrange("(W m g) -> m W g", W=NCHUNK, g=NG),
                    in_=out_sb,
                )

            # 3-deep software pipeline: transposes(b) | stage2(b-2) | stage1(b-1)
            # stage2 is emitted before stage1 so the PE stream has ready work
            # (stage2 of b-2) while the DVE works through stage1 of b-1.
            def emit_all():
                for b in range(S + 2):
                    if 0 <= b - 1 < S:
                        emit_stage1_dve(b - 1)
                    if b < S:
                        emit_load_transpose(b)
                    if 0 <= b - 2 < S:
                        emit_stage2(b - 2)
                    if 0 <= b - 1 < S:
                        emit_stage1_rest(b - 1)

            if repeat == 1:
                emit_all()
            elif repeat < 0:  # dynamic loop (barrier per iteration)
                with tc.For_i(0, -repeat, 1):
                    emit_all()
            else:  # fully unrolled
                for _rep in range(repeat):
                    emit_all()

    nc.compile()
    return nc


USE_Y = True


def _host_inputs(xs):
    """Global (all-core concatenated) input arrays keyed by dram tensor name."""
    feed = {"x": xs}
    if USE_Y:
        feed["ident"] = np.tile(np.eye(P, dtype=np.float32), (N_CORES, 1))
        s_idx = np.arange(5)[:, None]
        p_idx = np.arange(P)[None, :]
        cmap = (128 * s_idx + p_idx) % CHI  # [5, P]
        gmap = (128 * s_idx + p_idx) // CHI  # [5, P]
        ind1 = (np.arange(CHI)[None, :, None] == cmap[:, None, :]).astype(np.float32)
        ind2 = (np.arange(32)[None, None, :] == gmap[:, :, None]).astype(np.float32)
        feed["ind1"] = np.tile(ind1, (N_CORES, 1, 1))
        feed["ind2"] = np.tile(ind2, (N_CORES, 1, 1))
    return feed


def _get_nc():
    if "nc" not in _CACHE:
        _CACHE["nc"] = _build_nc_y() if USE_Y else _build_nc()
    return _CACHE["nc"]


def _get_runner():
    if "runner" not in _CACHE:
        run, sharded, mesh, body = _make_runner(_get_nc())
        _CACHE["sharded"] = sharded
        _CACHE["mesh"] = mesh
        _CACHE["body"] = body
        _CACHE["runner"] = run
    return _CACHE["runner"]


def _make_runner(nc):
    """Compile once and return f(x_global[64, CHI*D]) -> y_global[64, D].

    Mirrors concourse.bass2jax.run_bass_via_pjrt but caches the jitted
    executable so repeated kernel() calls don't re-trace/re-compile.
    """
    import jax
    from jax.sharding import Mesh, PartitionSpec
    from jax.experimental.shard_map import shard_map
    from concourse import bass2jax, mybir

    bass2jax.install_neuronx_cc_hook()

    partition_name = (
        nc.partition_id_tensor.name if nc.partition_id_tensor else None
    )
    in_names = []
    out_names = []
    out_avals = []
    zero_outs = []
    for alloc in nc.m.functions[0].allocations:
        if not isinstance(alloc, mybir.MemoryLocationSet):
            continue
        name = alloc.memorylocations[0].name
        if alloc.kind == "ExternalInput":
            if name != partition_name:
                in_names.append(name)
        elif alloc.kind == "ExternalOutput":
            shape = tuple(alloc.tensor_shape)
            dtype = mybir.dt.np(alloc.dtype)
            out_avals.append(jax.core.ShapedArray(shape, dtype))
            out_names.append(name)
            zero_outs.append(np.zeros(shape, dtype))
    n_params = len(in_names)
    n_outs = len(out_avals)
    in_names.extend(out_names)
    donate = tuple(range(n_params, n_params + n_outs))

    def _body(*args):
        operands = list(args)
        if partition_name is not None:
            operands.append(bass2jax.partition_id_tensor())
            in_full = tuple(in_names) + (partition_name,)
        else:
            in_full = tuple(in_names)
        outs = bass2jax._bass_exec_p.bind(
            *operands,
            out_avals=tuple(out_avals),
            in_names=in_full,
            out_names=tuple(out_names),
            lowering_input_output_aliases=(),
            sim_require_finite=True,
            sim_require_nnan=True,
            nc=nc,
        )
        return tuple(outs)

    devices = jax.devices()[:N_CORES]
    mesh = Mesh(np.asarray(devices), ("core",))
    in_specs = (PartitionSpec("core"),) * (n_params + n_outs)
    out_specs = (PartitionSpec("core"),) * len(out_names)
    sharded = jax.jit(
        shard_map(
            _body, mesh=mesh, in_specs=in_specs, out_specs=out_specs, check_rep=False
        ),
        donate_argnums=donate,
        keep_unused=True,
    )

    param_names = in_names[:n_params]
    _CACHE["param_names"] = param_names

    def run(xs):
        feed = _host_inputs(xs)
        args = [feed[n] for n in param_names]
        concat_zeros = [
            np.zeros((N_CORES * z.shape[0], *z.shape[1:]), z.dtype) for z in zero_outs
        ]
        return sharded(*args, *concat_zeros)[0]

    return run, sharded, mesh, _body


def kernel(**inputs):
    x = np.ascontiguousarray(np.asarray(inputs["x"], dtype=np.float32))
    assert x.shape == (B, CHI, 64, 32, 32), x.shape
    xs = x.reshape(B, CHI * D)
    run = _get_runner()
    last_err = None
    for _attempt in range(3):
        try:
            out = np.asarray(run(xs))
            break
        except Exception as e:  # transient NRT device errors: retry
            last_err = e
    else:
        raise last_err
    return out.reshape(B, 64, 32, 32)



# revision 3
# speedup vs baseline: 488.3741x; 488.3741x over previous
"""ConvLSTM attention pooling kernel for 8 Trainium2 NeuronCores.

Reference computation (per sample b):
    frames = x[b].reshape(chi, D)            # D = C*H*W = 65536, chi = 20
    scores = frames @ frames[-1] / chi       # [chi]
    alpha  = softmax(scores)                 # [chi]
    y      = x[b].reshape(D, chi) @ alpha    # [D]  (interleaved view!)

Sharding: pure data-parallel over batch B=64 -> 8 samples per core.

Builders:
  - _build_nc_t : current default. Input is host-side converted to bf16 and
                  pre-transposed per sample to want[q, f] = flat[128*f + q]
                  ([128, 10240] per sample), so one contiguous DMA loads a
                  layout that serves both stages:
                    * stage 1 (scores): frame c is the contiguous column
                      slice [512c, 512c+512) -> 19 fused multiply+reduce DVE
                      ops against frame 19 (bf16, 2x mode) + 1 ACT square.
                    * stage 2 (weighted sum over the interleaved view): the
                      data is the MOVING matmul operand (streams at 2.4 GHz);
                      the stationary operands are 5 small per-sample "alpha
                      scatter" tiles [128, 32]. Column block n of phase s
                      covers flat elements 640n + 128s + q, which land in
                      output row 32n + (128s+q)//20 with weight
                      alpha[(128s+q)%20] -- exactly the scatter tiles.
                      PSUM [128, 512] collects y in a (t, g, n') permuted
                      order that the host untangles with one cheap reshape.
  - _build_nc_y : previous PE-transpose fp32 architecture (fallback).
  - _build_nc   : two-HBM-read fp32 version (fallback).
"""

import numpy as np
import ml_dtypes

BF16 = ml_dtypes.bfloat16

B = 64
CHI = 20
D = 64 * 32 * 32  # 65536
N_CORES = 8
S = B // N_CORES  # samples per core
P = 128
F = D // P  # 512
NB = CHI * F  # 10240 columns per sample in want layout

KIND = "t"  # "t" (bf16 transposed-load) | "y" (PE-transpose) | "2r" (two-read)

_CACHE = {}


def _build_nc_t(repeat=1):
    import concourse.bacc as bacc
    import concourse.tile as tile
    from concourse import mybir

    f32 = mybir.dt.float32
    bf16 = mybir.dt.bfloat16
    nc = bacc.Bacc("TRN2", target_bir_lowering=False, debug=False)
    x_d = nc.dram_tensor("x", [S, P, NB], bf16, kind="ExternalInput").ap()
    ind1_d = nc.dram_tensor("ind1", [5, CHI, P], f32, kind="ExternalInput").ap()
    ind2_d = nc.dram_tensor("ind2", [5, P, 32], bf16, kind="ExternalInput").ap()
    y_d = nc.dram_tensor("y", [S, D], f32, kind="ExternalOutput").ap()

    NT = 4  # psum partition groups of 32 (output rows per 640-block)
    NPH = 5  # phases (columns mod 5)

    with tile.TileContext(nc) as tc:
        with (
            tc.tile_pool(name="want", bufs=3) as want_pool,
            tc.tile_pool(name="scratch", bufs=2) as s_pool,
            tc.tile_pool(name="small", bufs=6) as sm_pool,
            tc.tile_pool(name="scat", bufs=3) as scat_pool,
            tc.tile_pool(name="osb", bufs=3) as osb_pool,
            tc.tile_pool(name="singles", bufs=1) as ones_pool,
            tc.tile_pool(name="pss", bufs=2, space="PSUM") as pss_pool,
            tc.tile_pool(name="pso", bufs=2, space="PSUM") as pso_pool,
        ):
            inv_chi_col = ones_pool.tile([P, 1], f32)
            nc.vector.memset(inv_chi_col, 1.0 / CHI)
            ident1 = ones_pool.tile([1, 1], f32)
            nc.vector.memset(ident1, 1.0)
            ind1 = ones_pool.tile([CHI, NPH, P], f32)
            nc.sync.dma_start(out=ind1, in_=ind1_d.rearrange("s c p -> c s p"))
            ind2 = ones_pool.tile([P, NPH, 32], bf16)
            nc.sync.dma_start(out=ind2, in_=ind2_d.rearrange("s p g -> p s g"))

            wants = [None] * S
            scats = [None] * S

            def emit_load(b):
                want = want_pool.tile([P, NB], bf16)
                wants[b] = want
                half = NB // 2
                nc.sync.dma_start(out=want[:, :half], in_=x_d[b][:, :half])
                nc.sync.dma_start(out=want[:, half:], in_=x_d[b][:, half:])

            def emit_stage1(b):
                """Scores + softmax + alpha scatter tiles for sample b."""
                want = wants[b]
                f19 = want[:, (CHI - 1) * F : CHI * F]
                partials = sm_pool.tile([P, CHI], f32)
                sq = s_pool.tile([P, F], bf16, tag="scr")
                nc.scalar.activation(
                    out=sq,
                    in_=f19,
                    func=mybir.ActivationFunctionType.Square,
                    accum_out=partials[:, CHI - 1 : CHI],
                )
                for c in range(CHI - 1):
                    scratch = s_pool.tile([P, F], bf16, tag="scr")
                    nc.vector.scalar_tensor_tensor(
                        out=scratch,
                        in0=want[:, c * F : (c + 1) * F],
                        scalar=1.0,
                        in1=f19,
                        op0=mybir.AluOpType.mult,
                        op1=mybir.AluOpType.mult,
                        accum_out=partials[:, c : c + 1],
                    )

                soft = pss_pool.tile([P, 48], f32)  # one psum bank, sliced
                s_psum = soft[0:1, 0:CHI]
                a_psum = soft[0:CHI, 24:25]
                a_pat = soft[:, 32:40]
                nc.tensor.matmul(s_psum, inv_chi_col, partials, start=True, stop=True)
                scores = sm_pool.tile([1, CHI], f32)
                nc.vector.tensor_copy(out=scores, in_=s_psum)

                neg_mx = sm_pool.tile([1, 1], f32)
                nc.vector.tensor_reduce(
                    out=neg_mx,
                    in_=scores,
                    axis=mybir.AxisListType.X,
                    op=mybir.AluOpType.max,
                    negate=True,
                )
                exps = sm_pool.tile([1, CHI], f32)
                sumexp = sm_pool.tile([1, 1], f32)
                nc.scalar.activation(
                    out=exps,
                    in_=scores,
                    func=mybir.ActivationFunctionType.Exp,
                    bias=neg_mx[:, 0:1],
                    scale=1.0,
                    accum_out=sumexp,
                )
                rsum = sm_pool.tile([1, 1], f32)
                nc.vector.reciprocal(rsum, sumexp)
                alpha = sm_pool.tile([1, CHI], f32)
                nc.vector.tensor_scalar_mul(alpha, exps, rsum)

                nc.tensor.transpose(a_psum, alpha, ident1)
                a_one = sm_pool.tile([CHI, 1], f32)
                nc.scalar.copy(out=a_one, in_=a_psum)

                # a_pat[:, s] = ind1_s.T @ alpha_col = alpha[(128s+q) % 20]
                for s in range(NPH):
                    nc.tensor.matmul(
                        a_pat[:, s : s + 1], ind1[:, s, :], a_one,
                        start=True, stop=True,
                    )
                # scat_s[q, g] = ind2_s[q, g] * a_pat[q, s]
                scat = scat_pool.tile([P, NPH, 32], bf16)
                scats[b] = scat
                for s in range(NPH):
                    nc.vector.tensor_scalar_mul(
                        scat[:, s, :], ind2[:, s, :], a_pat[:, s : s + 1]
                    )

            def emit_stage2(b):
                """Weighted interleaved sum on PE + store for sample b."""
                want = wants[b]
                scat = scats[b]
                # want_ph[q, s, n] = want[q, 5n + s]
                want_ph = want.rearrange("q (n five) -> q five n", five=NPH)
                ob = pso_pool.tile([P, F], f32)
                for t in range(NT):
                    for s in range(NPH):
                        nc.tensor.matmul(
                            ob[32 * t : 32 * (t + 1), :],
                            scat[:, s, :],
                            want_ph[:, s, F * t : F * (t + 1)],
                            start=(s == 0),
                            stop=(s == NPH - 1),
                            tile_position=(0, 32 * t),
                        )
                out_sb = osb_pool.tile([P, F], f32)
                nc.scalar.copy(out=out_sb, in_=ob)
                nc.sync.dma_start(
                    out=y_d[b].rearrange("(p f) -> p f", p=P), in_=out_sb
                )

            def emit_all():
                for b in range(S + 1):
                    if b < S:
                        emit_load(b)
                    if 0 <= b - 1:
                        emit_stage2(b - 1)
                    if b < S:
                        emit_stage1(b)

            if repeat == 1:
                emit_all()
            else:
                for _rep in range(repeat):
                    emit_all()

    nc.compile()
    return nc


def _build_nc(repeat=1):
    import concourse.bacc as bacc
    import concourse.tile as tile
    from concourse import mybir

    f32 = mybir.dt.float32
    nc = bacc.Bacc("TRN2", target_bir_lowering=False, debug=False)
    x_d = nc.dram_tensor("x", [S, CHI * D], f32, kind="ExternalInput").ap()
    y_d = nc.dram_tensor("y", [S, D], f32, kind="ExternalOutput").ap()

    with tile.TileContext(nc) as tc:
        with (
            tc.tile_pool(name="t1", bufs=2) as t1_pool,
            tc.tile_pool(name="t2", bufs=2) as t2_pool,
            tc.tile_pool(name="scratch", bufs=4) as s_pool,
            tc.tile_pool(name="small", bufs=4) as sm_pool,
            tc.tile_pool(name="outp", bufs=2) as o_pool,
            tc.tile_pool(name="singles", bufs=1) as ones_pool,
            tc.tile_pool(name="psum", bufs=2, space="PSUM") as p_pool,
        ):
            inv_chi_col = ones_pool.tile([P, 1], f32)
            nc.vector.memset(inv_chi_col, 1.0 / CHI)
            ones_row = ones_pool.tile([1, P], f32)
            nc.vector.memset(ones_row, 1.0)

            for _rep in range(repeat):
              for b in range(S):
                u = x_d[b]
                t1 = t1_pool.tile([P, CHI, F], f32)
                nc.sync.dma_start(
                    out=t1, in_=u.rearrange("(c p f) -> p c f", p=P, f=F)
                )
                t2 = t2_pool.tile([P, F, CHI], f32)
                nc.sync.dma_start(
                    out=t2, in_=u.rearrange("(p f c) -> p f c", p=P, c=CHI)
                )

                partials = sm_pool.tile([P, CHI], f32)
                scratch = s_pool.tile([P, F], f32)
                for c in range(CHI):
                    nc.vector.scalar_tensor_tensor(
                        out=scratch,
                        in0=t1[:, c, :],
                        scalar=1.0,
                        in1=t1[:, CHI - 1, :],
                        op0=mybir.AluOpType.mult,
                        op1=mybir.AluOpType.mult,
                        accum_out=partials[:, c : c + 1],
                    )

                s_psum = p_pool.tile([1, CHI], f32)
                nc.tensor.matmul(s_psum, inv_chi_col, partials, start=True, stop=True)
                scores = sm_pool.tile([1, CHI], f32)
                nc.vector.tensor_copy(out=scores, in_=s_psum)

                neg_mx = sm_pool.tile([1, 1], f32)
                nc.vector.tensor_reduce(
                    out=neg_mx,
                    in_=scores,
                    axis=mybir.AxisListType.X,
                    op=mybir.AluOpType.max,
                    negate=True,
                )
                exps = sm_pool.tile([1, CHI], f32)
                sumexp = sm_pool.tile([1, 1], f32)
                nc.scalar.activation(
                    out=exps,
                    in_=scores,
                    func=mybir.ActivationFunctionType.Exp,
                    bias=neg_mx[:, 0:1],
                    scale=1.0,
                    accum_out=sumexp,
                )
                rsum = sm_pool.tile([1, 1], f32)
                nc.vector.reciprocal(rsum, sumexp)
                alpha = sm_pool.tile([1, CHI], f32)
                nc.vector.tensor_scalar_mul(alpha, exps, rsum)

                b_psum = p_pool.tile([P, CHI], f32)
                nc.tensor.matmul(b_psum, ones_row, alpha, start=True, stop=True)
                alpha_bc = sm_pool.tile([P, CHI], f32)
                nc.scalar.copy(out=alpha_bc, in_=b_psum)

                out_t = o_pool.tile([P, F], f32)
                nc.vector.tensor_scalar_mul(out_t, t2[:, :, 0], alpha_bc[:, 0:1])
                for c in range(1, CHI):
                    nc.vector.scalar_tensor_tensor(
                        out=out_t,
                        in0=t2[:, :, c],
                        scalar=alpha_bc[:, c : c + 1],
                        in1=out_t,
                        op0=mybir.AluOpType.mult,
                        op1=mybir.AluOpType.add,
                    )

                nc.sync.dma_start(
                    out=y_d[b].rearrange("(p f) -> p f", p=P), in_=out_t
                )

    nc.compile()
    return nc


def _build_nc_y(ablate=(), repeat=1):
    """Single-HBM-read fp32 architecture (PE transpose + scatter stationary)."""
    import concourse.bacc as bacc
    import concourse.tile as tile
    from concourse import mybir

    f32 = mybir.dt.float32
    nc = bacc.Bacc("TRN2", target_bir_lowering=False, debug=False)
    x_d = nc.dram_tensor("x", [S, CHI * D], f32, kind="ExternalInput").ap()
    ident_d = nc.dram_tensor("ident", [P, P], f32, kind="ExternalInput").ap()
    ind1_d = nc.dram_tensor("ind1", [5, CHI, P], f32, kind="ExternalInput").ap()
    ind2_d = nc.dram_tensor("ind2", [5, P, 32], f32, kind="ExternalInput").ap()
    y_d = nc.dram_tensor("y", [S, D], f32, kind="ExternalOutput").ap()

    NW = 5
    NG = 32
    NB_ = 80
    NCHUNK = 16

    with tile.TileContext(nc) as tc:
        with (
            tc.tile_pool(name="nt", bufs=5) as nt_pool,
            tc.tile_pool(name="want", bufs=3) as want_pool,
            tc.tile_pool(name="scratch", bufs=4) as s_pool,
            tc.tile_pool(name="small", bufs=6) as sm_pool,
            tc.tile_pool(name="rhs", bufs=3) as rhs_pool,
            tc.tile_pool(name="osb", bufs=3) as osb_pool,
            tc.tile_pool(name="singles", bufs=1) as ones_pool,
            tc.tile_pool(name="pst", bufs=5, space="PSUM") as pst_pool,
            tc.tile_pool(name="pss", bufs=1, space="PSUM") as pss_pool,
            tc.tile_pool(name="pso", bufs=2, space="PSUM") as pso_pool,
        ):
            inv_chi_col = ones_pool.tile([P, 1], f32)
            nc.vector.memset(inv_chi_col, 1.0 / CHI)
            ident = ones_pool.tile([P, P], f32)
            nc.sync.dma_start(out=ident, in_=ident_d)
            ind1 = ones_pool.tile([CHI, 5, P], f32)
            nc.sync.dma_start(out=ind1, in_=ind1_d.rearrange("s c p -> c s p"))
            ind2 = ones_pool.tile([P, 5, 32], f32)
            nc.sync.dma_start(out=ind2, in_=ind2_d.rearrange("s p g -> p s g"))

            HB = NB_ // 4
            wants = [None] * S
            rhss = [None] * S

            def emit_load_transpose(b):
                u = x_d[b].rearrange("(p q) -> p q", p=P)
                want = want_pool.tile([P, CHI * F], f32)
                wants[b] = want
                want_v = want.rearrange("p (pp j) -> p j pp", j=NB_)
                for h in range(4):
                    nt = nt_pool.tile([P, HB * P], f32)
                    nc.sync.dma_start(out=nt, in_=u[:, h * HB * P : (h + 1) * HB * P])
                    for jg in range(HB // 4):
                        if "tc" in ablate:
                            break
                        ps = pst_pool.tile([P, 4, P], f32)
                        for dj in range(4):
                            j = jg * 4 + dj
                            nc.tensor.transpose(
                                ps[:, dj, :], nt[:, j * P : (j + 1) * P], ident
                            )
                        jga = h * (HB // 4) + jg
                        dst = want_v[:, jga * 4 : (jga + 1) * 4, :]
                        if jga % 10 == 0:
                            nc.vector.tensor_copy(out=dst, in_=ps)
                        else:
                            nc.scalar.copy(out=dst, in_=ps)

            partials_arr = [None] * S

            def emit_stage1_dve(b):
                want = wants[b]
                wv = want.rearrange("p (c e) -> p c e", c=CHI)
                partials = sm_pool.tile([P, CHI], f32)
                scratch = s_pool.tile([P, F], f32)
                sq = s_pool.tile([P, F], f32)
                if "s1" in ablate:
                    nc.vector.memset(partials, 0.05)
                if "s1" not in ablate: nc.scalar.activation(
                    out=sq,
                    in_=wv[:, CHI - 1, :],
                    func=mybir.ActivationFunctionType.Square,
                    accum_out=partials[:, CHI - 1 : CHI],
                )
                for c in range(CHI - 1):
                    if "s1" in ablate: break
                    nc.vector.scalar_tensor_tensor(
                        out=scratch,
                        in0=wv[:, c, :],
                        scalar=1.0,
                        in1=wv[:, CHI - 1, :],
                        op0=mybir.AluOpType.mult,
                        op1=mybir.AluOpType.mult,
                        accum_out=partials[:, c : c + 1],
                    )

                partials_arr[b] = partials

            def emit_stage1_rest(b):
                partials = partials_arr[b]
                soft = pss_pool.tile([P, 48], f32)
                s_psum = soft[0:1, 0:CHI]
                a_psum = soft[0:CHI, 24:25]
                a_pat = soft[:, 32:40]
                nc.tensor.matmul(s_psum, inv_chi_col, partials, start=True, stop=True)
                scores = sm_pool.tile([1, CHI], f32)
                nc.vector.tensor_copy(out=scores, in_=s_psum)

                neg_mx = sm_pool.tile([1, 1], f32)
                nc.vector.tensor_reduce(
                    out=neg_mx,
                    in_=scores,
                    axis=mybir.AxisListType.X,
                    op=mybir.AluOpType.max,
                    negate=True,
                )
                exps = sm_pool.tile([1, CHI], f32)
                sumexp = sm_pool.tile([1, 1], f32)
                nc.scalar.activation(
                    out=exps,
                    in_=scores,
                    func=mybir.ActivationFunctionType.Exp,
                    bias=neg_mx[:, 0:1],
                    scale=1.0,
                    accum_out=sumexp,
                )
                rsum = sm_pool.tile([1, 1], f32)
                nc.vector.reciprocal(rsum, sumexp)
                alpha = sm_pool.tile([1, CHI], f32)
                nc.vector.tensor_scalar_mul(alpha, exps, rsum)

                nc.tensor.transpose(a_psum, alpha, ident[:1, :1])
                a_one = sm_pool.tile([CHI, 1], f32)
                nc.scalar.copy(out=a_one, in_=a_psum)

                for s in range(NW):
                    nc.tensor.matmul(
                        a_pat[:, s : s + 1], ind1[:, s, :], a_one,
                        start=True, stop=True,
                    )
                rhs = rhs_pool.tile([P, NW, NG], f32)
                rhss[b] = rhs
                for s in range(NW):
                    nc.vector.tensor_scalar_mul(
                        rhs[:, s, :], ind2[:, s, :], a_pat[:, s : s + 1]
                    )

            def emit_stage2(b):
                want = wants[b]
                rhs = rhss[b]
                ob = pso_pool.tile([P, NCHUNK, NG], f32)
                want_w = want.rearrange("p (m s) -> p s m", s=NW)
                for W in range(NCHUNK):
                    if "s2" in ablate:
                        nc.tensor.matmul(ob[:, W, :], wants[b][:, :P], rhs[:, 0, :], start=True, stop=True)
                        continue
                    for s in range(NW):
                        nc.tensor.matmul(
                            ob[:, W, :],
                            want_w[:, s, W * P : (W + 1) * P],
                            rhs[:, s, :],
                            start=(s == 0),
                            stop=(s == NW - 1),
                        )
                out_sb = osb_pool.tile([P, NCHUNK, NG], f32)
                nc.scalar.copy(out=out_sb, in_=ob)
                nc.sync.dma_start(
                    out=y_d[b].rearrange("(W m g) -> m W g", W=NCHUNK, g=NG),
                    in_=out_sb,
                )

            def emit_all():
                for b in range(S + 2):
                    if 0 <= b - 1 < S:
                        emit_stage1_dve(b - 1)
                    if b < S:
                        emit_load_transpose(b)
                    if 0 <= b - 2 < S:
                        emit_stage2(b - 2)
                    if 0 <= b - 1 < S:
                        emit_stage1_rest(b - 1)

            if repeat == 1:
                emit_all()
            elif repeat < 0:
                with tc.For_i(0, -repeat, 1):
                    emit_all()
            else:
                for _rep in range(repeat):
                    emit_all()

    nc.compile()
    return nc


def _scatter_maps():
    s_idx = np.arange(5)[:, None]
    p_idx = np.arange(P)[None, :]
    cmap = (128 * s_idx + p_idx) % CHI  # [5, P]
    gmap = (128 * s_idx + p_idx) // CHI  # [5, P]
    ind1 = (np.arange(CHI)[None, :, None] == cmap[:, None, :]).astype(np.float32)
    ind2 = (np.arange(32)[None, None, :] == gmap[:, :, None]).astype(np.float32)
    return ind1, ind2


def _host_inputs(xs):
    """Global (all-core concatenated) input arrays keyed by dram tensor name.

    xs: [B, CHI*D] float32 (row-major flat samples).
    """
    if KIND == "t":
        # want[b, q, f] = xs[b, 128*f + q], as bf16
        xt = np.ascontiguousarray(
            xs.reshape(B, NB, P).transpose(0, 2, 1).astype(BF16)
        )
        ind1, ind2 = _scatter_maps()
        return {
            "x": xt,
            "ind1": np.tile(ind1, (N_CORES, 1, 1)),
            "ind2": np.tile(ind2.astype(BF16), (N_CORES, 1, 1)),
        }
    feed = {"x": xs}
    if KIND == "y":
        ind1, ind2 = _scatter_maps()
        feed["ident"] = np.tile(np.eye(P, dtype=np.float32), (N_CORES, 1))
        feed["ind1"] = np.tile(ind1, (N_CORES, 1, 1))
        feed["ind2"] = np.tile(ind2, (N_CORES, 1, 1))
    return feed


def _unpermute_y(y_raw):
    """Invert the device output permutation of the "t" kernel.

    Device wrote y_raw[b, (32t+g)*512 + n] = y[b, 16384t + 32n + g].
    """
    if KIND != "t":
        return y_raw
    return np.ascontiguousarray(
        y_raw.reshape(B, 4, 32, 512).transpose(0, 1, 3, 2)
    ).reshape(B, D)


def _get_nc():
    if "nc" not in _CACHE:
        if KIND == "t":
            _CACHE["nc"] = _build_nc_t()
        elif KIND == "y":
            _CACHE["nc"] = _build_nc_y()
        else:
            _CACHE["nc"] = _build_nc()
    return _CACHE["nc"]


def _get_runner():
    if "runner" not in _CACHE:
        run, sharded, mesh, body = _make_runner(_get_nc())
        _CACHE["sharded"] = sharded
        _CACHE["mesh"] = mesh
        _CACHE["body"] = body
        _CACHE["runner"] = run
    return _CACHE["runner"]


def _make_runner(nc):
    """Compile once and return f(x_global[64, CHI*D]) -> y_global[64, D]."""
    import jax
    from jax.sharding import Mesh, PartitionSpec
    from jax.experimental.shard_map import shard_map
    from concourse import bass2jax, mybir

    bass2jax.install_neuronx_cc_hook()

    partition_name = (
        nc.partition_id_tensor.name if nc.partition_id_tensor else None
    )
    in_names = []
    out_names = []
    out_avals = []
    zero_outs = []
    for alloc in nc.m.functions[0].allocations:
        if not isinstance(alloc, mybir.MemoryLocationSet):
            continue
        name = alloc.memorylocations[0].name
        if alloc.kind == "ExternalInput":
            if name != partition_name:
                in_names.append(name)
        elif alloc.kind == "ExternalOutput":
            shape = tuple(alloc.tensor_shape)
            dtype = mybir.dt.np(alloc.dtype)
            out_avals.append(jax.core.ShapedArray(shape, dtype))
            out_names.append(name)
            zero_outs.append(np.zeros(shape, dtype))
    n_params = len(in_names)
    n_outs = len(out_avals)
    in_names.extend(out_names)
    donate = tuple(range(n_params, n_params + n_outs))

    def _body(*args):
        operands = list(args)
        if partition_name is not None:
            operands.append(bass2jax.partition_id_tensor())
            in_full = tuple(in_names) + (partition_name,)
        else:
            in_full = tuple(in_names)
        outs = bass2jax._bass_exec_p.bind(
            *operands,
            out_avals=tuple(out_avals),
            in_names=in_full,
            out_names=tuple(out_names),
            lowering_input_output_aliases=(),
            sim_require_finite=True,
            sim_require_nnan=True,
            nc=nc,
        )
        return tuple(outs)

    devices = jax.devices()[:N_CORES]
    mesh = Mesh(np.asarray(devices), ("core",))
    in_specs = (PartitionSpec("core"),) * (n_params + n_outs)
    out_specs = (PartitionSpec("core"),) * len(out_names)
    sharded = jax.jit(
        shard_map(
            _body, mesh=mesh, in_specs=in_specs, out_specs=out_specs, check_rep=False
        ),
        donate_argnums=donate,
        keep_unused=True,
    )

    param_names = in_names[:n_params]
    _CACHE["param_names"] = param_names

    def run(xs):
        feed = _host_inputs(xs)
        args = [feed[n] for n in param_names]
        concat_zeros = [
            np.zeros((N_CORES * z.shape[0], *z.shape[1:]), z.dtype) for z in zero_outs
        ]
        return sharded(*args, *concat_zeros)[0]

    return run, sharded, mesh, _body


def kernel(**inputs):
    x = np.ascontiguousarray(np.asarray(inputs["x"], dtype=np.float32))
    assert x.shape == (B, CHI, 64, 32, 32), x.shape
    xs = x.reshape(B, CHI * D)
    run = _get_runner()
    last_err = None
    for _attempt in range(3):
        try:
            out = np.asarray(run(xs))
            break
        except Exception as e:  # transient NRT device errors: retry
            last_err = e
    else:
        raise last_err
    out = _unpermute_y(out)
    return out.reshape(B, 64, 32, 32)


# revision 6
# speedup vs baseline: 596.5083x; 1.2214x over previous
"""ConvLSTM attention pooling kernel for 8 Trainium2 NeuronCores.

Reference computation (per sample b):
    frames = x[b].reshape(chi, D)            # D = C*H*W = 65536, chi = 20
    scores = frames @ frames[-1] / chi       # [chi]
    alpha  = softmax(scores)                 # [chi]
    y      = x[b].reshape(D, chi) @ alpha    # [D]  (interleaved view!)

Sharding: pure data-parallel over batch B=64 -> 8 samples per core.

Builders:
  - _build_nc_t : current default. Input is host-side converted to bf16 and
                  pre-transposed per sample to want[q, f] = flat[128*f + q]
                  ([128, 10240] per sample), so one contiguous DMA loads a
                  layout that serves both stages:
                    * stage 1 (scores): frame c is the contiguous column
                      slice [512c, 512c+512) -> 19 fused multiply+reduce DVE
                      ops against frame 19 (bf16, 2x mode) + 1 ACT square.
                    * stage 2 (weighted sum over the interleaved view): the
                      data is the MOVING matmul operand (streams at 2.4 GHz);
                      the stationary operands are 5 small per-sample "alpha
                      scatter" tiles [128, 32]. Column block n of phase s
                      covers flat elements 640n + 128s + q, which land in
                      output row 32n + (128s+q)//20 with weight
                      alpha[(128s+q)%20] -- exactly the scatter tiles.
                      PSUM [128, 512] collects y in a (t, g, n') permuted
                      order that the host untangles with one cheap reshape.
  - _build_nc_y : previous PE-transpose fp32 architecture (fallback).
  - _build_nc   : two-HBM-read fp32 version (fallback).
"""

import numpy as np
import ml_dtypes

BF16 = ml_dtypes.bfloat16

B = 64
CHI = 20
D = 64 * 32 * 32  # 65536
N_CORES = 8
S = B // N_CORES  # samples per core
P = 128
F = D // P  # 512
NB = CHI * F  # 10240 columns per sample in want layout

KIND = "t"  # "t" (bf16 transposed-load) | "y" (PE-transpose) | "2r" (two-read)

_CACHE = {}


def _build_nc_t(repeat=1):
    import concourse.bacc as bacc
    import concourse.tile as tile
    from concourse import mybir

    f32 = mybir.dt.float32
    bf16 = mybir.dt.bfloat16
    nc = bacc.Bacc("TRN2", target_bir_lowering=False, debug=False)
    x_d = nc.dram_tensor("x", [S, P, NB], bf16, kind="ExternalInput").ap()
    ind1_d = nc.dram_tensor("ind1", [5, CHI, P], f32, kind="ExternalInput").ap()
    ind2_d = nc.dram_tensor("ind2", [5, P, 32], bf16, kind="ExternalInput").ap()
    y_d = nc.dram_tensor("y", [S, D], f32, kind="ExternalOutput").ap()

    NT = 4  # psum partition groups of 32 (output rows per 640-block)
    NPH = 5  # phases (columns mod 5)

    with tile.TileContext(nc) as tc:
        with (
            tc.tile_pool(name="wlo", bufs=3) as wlo_pool,
            tc.tile_pool(name="whi", bufs=3) as whi_pool,
            tc.tile_pool(name="scratch", bufs=2) as s_pool,
            tc.tile_pool(name="small", bufs=6) as sm_pool,
            tc.tile_pool(name="scat", bufs=3) as scat_pool,
            tc.tile_pool(name="osb", bufs=3) as osb_pool,
            tc.tile_pool(name="singles", bufs=1) as ones_pool,
            tc.tile_pool(name="pss", bufs=2, space="PSUM") as pss_pool,
            tc.tile_pool(name="pso", bufs=2, space="PSUM") as pso_pool,
        ):
            inv_chi_col = ones_pool.tile([P, 1], f32)
            nc.vector.memset(inv_chi_col, 1.0 / CHI)
            ident1 = ones_pool.tile([1, 1], f32)
            nc.vector.memset(ident1, 1.0)
            ind1 = ones_pool.tile([CHI, NPH, P], f32)
            nc.sync.dma_start(out=ind1, in_=ind1_d.rearrange("s c p -> c s p"))
            ind2 = ones_pool.tile([P, NPH, 32], bf16)
            nc.sync.dma_start(out=ind2, in_=ind2_d.rearrange("s p g -> p s g"))

            wlos = [None] * S
            whis = [None] * S
            scats = [None] * S
            HALF = NB // 2  # 5120 columns = 10 frames per half

            def emit_load(b):
                # hi half (frames 10-19, incl. frame 19) loads first so the
                # score dot-products can start before the lo half lands
                whi = whi_pool.tile([P, HALF], bf16)
                wlo = wlo_pool.tile([P, HALF], bf16)
                whis[b] = whi
                wlos[b] = wlo
                q = HALF // 2
                nc.sync.dma_start(out=whi[:, q:], in_=x_d[b][:, HALF + q :])
                nc.sync.dma_start(out=whi[:, :q], in_=x_d[b][:, HALF : HALF + q])
                nc.sync.dma_start(out=wlo[:, :q], in_=x_d[b][:, :q])
                nc.sync.dma_start(out=wlo[:, q:], in_=x_d[b][:, q:HALF])

            def emit_stage1(b):
                """Scores + softmax + alpha scatter tiles for sample b."""
                wlo, whi = wlos[b], whis[b]
                f19 = whi[:, HALF - F :]
                partials = sm_pool.tile([P, CHI], f32)
                sq = s_pool.tile([P, F], bf16, tag="scr")
                nc.scalar.activation(
                    out=sq,
                    in_=f19,
                    func=mybir.ActivationFunctionType.Square,
                    accum_out=partials[:, CHI - 1 : CHI],
                )
                for c in list(range(10, CHI - 1)) + list(range(10)):
                    src = whi if c >= 10 else wlo
                    scratch = s_pool.tile([P, F], bf16, tag="scr")
                    nc.vector.scalar_tensor_tensor(
                        out=scratch,
                        in0=src[:, (c % 10) * F : (c % 10 + 1) * F],
                        scalar=1.0,
                        in1=f19,
                        op0=mybir.AluOpType.mult,
                        op1=mybir.AluOpType.mult,
                        accum_out=partials[:, c : c + 1],
                    )

                soft = pss_pool.tile([P, 48], f32)  # one psum bank, sliced
                s_psum = soft[0:1, 0:CHI]
                a_psum = soft[0:CHI, 24:25]
                a_pat = soft[:, 32:40]
                nc.tensor.matmul(s_psum, inv_chi_col, partials, start=True, stop=True)
                scores = sm_pool.tile([1, CHI], f32)
                nc.vector.tensor_copy(out=scores, in_=s_psum)

                neg_mx = sm_pool.tile([1, 1], f32)
                nc.vector.tensor_reduce(
                    out=neg_mx,
                    in_=scores,
                    axis=mybir.AxisListType.X,
                    op=mybir.AluOpType.max,
                    negate=True,
                )
                exps = sm_pool.tile([1, CHI], f32)
                sumexp = sm_pool.tile([1, 1], f32)
                nc.scalar.activation(
                    out=exps,
                    in_=scores,
                    func=mybir.ActivationFunctionType.Exp,
                    bias=neg_mx[:, 0:1],
                    scale=1.0,
                    accum_out=sumexp,
                )
                rsum = sm_pool.tile([1, 1], f32)
                nc.vector.reciprocal(rsum, sumexp)
                alpha = sm_pool.tile([1, CHI], f32)
                nc.vector.tensor_scalar_mul(alpha, exps, rsum)

                nc.tensor.transpose(a_psum, alpha, ident1)
                a_one = sm_pool.tile([CHI, 1], f32)
                nc.scalar.copy(out=a_one, in_=a_psum)

                # a_pat[:, s] = ind1_s.T @ alpha_col = alpha[(128s+q) % 20]
                for s in range(NPH):
                    nc.tensor.matmul(
                        a_pat[:, s : s + 1], ind1[:, s, :], a_one,
                        start=True, stop=True,
                    )
                # scat_s[q, g] = ind2_s[q, g] * a_pat[q, s], built on ACT so
                # the DVE FIFO isn't blocked behind the PE a_pat matmuls
                ap_sb = sm_pool.tile([P, 8], f32)
                nc.scalar.copy(out=ap_sb, in_=a_pat)
                scat = scat_pool.tile([P, NPH, 32], bf16)
                scats[b] = scat
                for s in range(NPH):
                    nc.scalar.activation(
                        out=scat[:, s, :],
                        in_=ind2[:, s, :],
                        func=mybir.ActivationFunctionType.Copy,
                        scale=ap_sb[:, s : s + 1],
                    )

            def emit_stage2(b):
                """Weighted interleaved sum on PE + store for sample b."""
                scat = scats[b]
                # per half: half_ph[q, s, n] = half[q, 5n + s], n in [0, 1024)
                lo_ph = wlos[b].rearrange("q (n five) -> q five n", five=NPH)
                hi_ph = whis[b].rearrange("q (n five) -> q five n", five=NPH)
                ob = pso_pool.tile([P, F], f32)
                # phase-outer so the 4 column-group matmuls of a phase can
                # overlap in the PE array (distinct col_grps)
                for s in range(NPH):
                    for t in range(NT):
                        src_ph = lo_ph if t < 2 else hi_ph
                        nc.tensor.matmul(
                            ob[32 * t : 32 * (t + 1), :],
                            scat[:, s, :],
                            src_ph[:, s, F * (t % 2) : F * (t % 2 + 1)],
                            start=(s == 0),
                            stop=(s == NPH - 1),
                            tile_position=(0, 32 * t),
                            skip_group_check=True,
                        )
                out_sb = osb_pool.tile([P, F], f32)
                nc.scalar.copy(out=out_sb, in_=ob)
                nc.sync.dma_start(
                    out=y_d[b].rearrange("(p f) -> p f", p=P), in_=out_sb
                )

            def emit_all():
                for b in range(S + 1):
                    if b < S:
                        emit_load(b)
                    if 0 <= b - 1:
                        emit_stage2(b - 1)
                    if b < S:
                        emit_stage1(b)

            if repeat == 1:
                emit_all()
            else:
                for _rep in range(repeat):
                    emit_all()

    nc.compile()
    return nc


def _build_nc(repeat=1):
    import concourse.bacc as bacc
    import concourse.tile as tile
    from concourse import mybir

    f32 = mybir.dt.float32
    nc = bacc.Bacc("TRN2", target_bir_lowering=False, debug=False)
    x_d = nc.dram_tensor("x", [S, CHI * D], f32, kind="ExternalInput").ap()
    y_d = nc.dram_tensor("y", [S, D], f32, kind="ExternalOutput").ap()

    with tile.TileContext(nc) as tc:
        with (
            tc.tile_pool(name="t1", bufs=2) as t1_pool,
            tc.tile_pool(name="t2", bufs=2) as t2_pool,
            tc.tile_pool(name="scratch", bufs=4) as s_pool,
            tc.tile_pool(name="small", bufs=4) as sm_pool,
            tc.tile_pool(name="outp", bufs=2) as o_pool,
            tc.tile_pool(name="singles", bufs=1) as ones_pool,
            tc.tile_pool(name="psum", bufs=2, space="PSUM") as p_pool,
        ):
            inv_chi_col = ones_pool.tile([P, 1], f32)
            nc.vector.memset(inv_chi_col, 1.0 / CHI)
            ones_row = ones_pool.tile([1, P], f32)
            nc.vector.memset(ones_row, 1.0)

            for _rep in range(repeat):
              for b in range(S):
                u = x_d[b]
                t1 = t1_pool.tile([P, CHI, F], f32)
                nc.sync.dma_start(
                    out=t1, in_=u.rearrange("(c p f) -> p c f", p=P, f=F)
                )
                t2 = t2_pool.tile([P, F, CHI], f32)
                nc.sync.dma_start(
                    out=t2, in_=u.rearrange("(p f c) -> p f c", p=P, c=CHI)
                )

                partials = sm_pool.tile([P, CHI], f32)
                scratch = s_pool.tile([P, F], f32)
                for c in range(CHI):
                    nc.vector.scalar_tensor_tensor(
                        out=scratch,
                        in0=t1[:, c, :],
                        scalar=1.0,
                        in1=t1[:, CHI - 1, :],
                        op0=mybir.AluOpType.mult,
                        op1=mybir.AluOpType.mult,
                        accum_out=partials[:, c : c + 1],
                    )

                s_psum = p_pool.tile([1, CHI], f32)
                nc.tensor.matmul(s_psum, inv_chi_col, partials, start=True, stop=True)
                scores = sm_pool.tile([1, CHI], f32)
                nc.vector.tensor_copy(out=scores, in_=s_psum)

                neg_mx = sm_pool.tile([1, 1], f32)
                nc.vector.tensor_reduce(
                    out=neg_mx,
                    in_=scores,
                    axis=mybir.AxisListType.X,
                    op=mybir.AluOpType.max,
                    negate=True,
                )
                exps = sm_pool.tile([1, CHI], f32)
                sumexp = sm_pool.tile([1, 1], f32)
                nc.scalar.activation(
                    out=exps,
                    in_=scores,
                    func=mybir.ActivationFunctionType.Exp,
                    bias=neg_mx[:, 0:1],
                    scale=1.0,
                    accum_out=sumexp,
                )
                rsum = sm_pool.tile([1, 1], f32)
                nc.vector.reciprocal(rsum, sumexp)
                alpha = sm_pool.tile([1, CHI], f32)
                nc.vector.tensor_scalar_mul(alpha, exps, rsum)

                b_psum = p_pool.tile([P, CHI], f32)
                nc.tensor.matmul(b_psum, ones_row, alpha, start=True, stop=True)
                alpha_bc = sm_pool.tile([P, CHI], f32)
                nc.scalar.copy(out=alpha_bc, in_=b_psum)

                out_t = o_pool.tile([P, F], f32)
                nc.vector.tensor_scalar_mul(out_t, t2[:, :, 0], alpha_bc[:, 0:1])
                for c in range(1, CHI):
                    nc.vector.scalar_tensor_tensor(
                        out=out_t,
                        in0=t2[:, :, c],
                        scalar=alpha_bc[:, c : c + 1],
                        in1=out_t,
                        op0=mybir.AluOpType.mult,
                        op1=mybir.AluOpType.add,
                    )

                nc.sync.dma_start(
                    out=y_d[b].rearrange("(p f) -> p f", p=P), in_=out_t
                )

    nc.compile()
    return nc


def _build_nc_y(ablate=(), repeat=1):
    """Single-HBM-read fp32 architecture (PE transpose + scatter stationary)."""
    import concourse.bacc as bacc
    import concourse.tile as tile
    from concourse import mybir

    f32 = mybir.dt.float32
    nc = bacc.Bacc("TRN2", target_bir_lowering=False, debug=False)
    x_d = nc.dram_tensor("x", [S, CHI * D], f32, kind="ExternalInput").ap()
    ident_d = nc.dram_tensor("ident", [P, P], f32, kind="ExternalInput").ap()
    ind1_d = nc.dram_tensor("ind1", [5, CHI, P], f32, kind="ExternalInput").ap()
    ind2_d = nc.dram_tensor("ind2", [5, P, 32], f32, kind="ExternalInput").ap()
    y_d = nc.dram_tensor("y", [S, D], f32, kind="ExternalOutput").ap()

    NW = 5
    NG = 32
    NB_ = 80
    NCHUNK = 16

    with tile.TileContext(nc) as tc:
        with (
            tc.tile_pool(name="nt", bufs=5) as nt_pool,
            tc.tile_pool(name="want", bufs=3) as want_pool,
            tc.tile_pool(name="scratch", bufs=4) as s_pool,
            tc.tile_pool(name="small", bufs=6) as sm_pool,
            tc.tile_pool(name="rhs", bufs=3) as rhs_pool,
            tc.tile_pool(name="osb", bufs=3) as osb_pool,
            tc.tile_pool(name="singles", bufs=1) as ones_pool,
            tc.tile_pool(name="pst", bufs=5, space="PSUM") as pst_pool,
            tc.tile_pool(name="pss", bufs=1, space="PSUM") as pss_pool,
            tc.tile_pool(name="pso", bufs=2, space="PSUM") as pso_pool,
        ):
            inv_chi_col = ones_pool.tile([P, 1], f32)
            nc.vector.memset(inv_chi_col, 1.0 / CHI)
            ident = ones_pool.tile([P, P], f32)
            nc.sync.dma_start(out=ident, in_=ident_d)
            ind1 = ones_pool.tile([CHI, 5, P], f32)
            nc.sync.dma_start(out=ind1, in_=ind1_d.rearrange("s c p -> c s p"))
            ind2 = ones_pool.tile([P, 5, 32], f32)
            nc.sync.dma_start(out=ind2, in_=ind2_d.rearrange("s p g -> p s g"))

            HB = NB_ // 4
            wants = [None] * S
            rhss = [None] * S

            def emit_load_transpose(b):
                u = x_d[b].rearrange("(p q) -> p q", p=P)
                want = want_pool.tile([P, CHI * F], f32)
                wants[b] = want
                want_v = want.rearrange("p (pp j) -> p j pp", j=NB_)
                for h in range(4):
                    nt = nt_pool.tile([P, HB * P], f32)
                    nc.sync.dma_start(out=nt, in_=u[:, h * HB * P : (h + 1) * HB * P])
                    for jg in range(HB // 4):
                        if "tc" in ablate:
                            break
                        ps = pst_pool.tile([P, 4, P], f32)
                        for dj in range(4):
                            j = jg * 4 + dj
                            nc.tensor.transpose(
                                ps[:, dj, :], nt[:, j * P : (j + 1) * P], ident
                            )
                        jga = h * (HB // 4) + jg
                        dst = want_v[:, jga * 4 : (jga + 1) * 4, :]
                        if jga % 10 == 0:
                            nc.vector.tensor_copy(out=dst, in_=ps)
                        else:
                            nc.scalar.copy(out=dst, in_=ps)

            partials_arr = [None] * S

            def emit_stage1_dve(b):
                want = wants[b]
                wv = want.rearrange("p (c e) -> p c e", c=CHI)
                partials = sm_pool.tile([P, CHI], f32)
                scratch = s_pool.tile([P, F], f32)
                sq = s_pool.tile([P, F], f32)
                if "s1" in ablate:
                    nc.vector.memset(partials, 0.05)
                if "s1" not in ablate: nc.scalar.activation(
                    out=sq,
                    in_=wv[:, CHI - 1, :],
                    func=mybir.ActivationFunctionType.Square,
                    accum_out=partials[:, CHI - 1 : CHI],
                )
                for c in range(CHI - 1):
                    if "s1" in ablate: break
                    nc.vector.scalar_tensor_tensor(
                        out=scratch,
                        in0=wv[:, c, :],
                        scalar=1.0,
                        in1=wv[:, CHI - 1, :],
                        op0=mybir.AluOpType.mult,
                        op1=mybir.AluOpType.mult,
                        accum_out=partials[:, c : c + 1],
                    )

                partials_arr[b] = partials

            def emit_stage1_rest(b):
                partials = partials_arr[b]
                soft = pss_pool.tile([P, 48], f32)
                s_psum = soft[0:1, 0:CHI]
                a_psum = soft[0:CHI, 24:25]
                a_pat = soft[:, 32:40]
                nc.tensor.matmul(s_psum, inv_chi_col, partials, start=True, stop=True)
                scores = sm_pool.tile([1, CHI], f32)
                nc.vector.tensor_copy(out=scores, in_=s_psum)

                neg_mx = sm_pool.tile([1, 1], f32)
                nc.vector.tensor_reduce(
                    out=neg_mx,
                    in_=scores,
                    axis=mybir.AxisListType.X,
                    op=mybir.AluOpType.max,
                    negate=True,
                )
                exps = sm_pool.tile([1, CHI], f32)
                sumexp = sm_pool.tile([1, 1], f32)
                nc.scalar.activation(
                    out=exps,
                    in_=scores,
                    func=mybir.ActivationFunctionType.Exp,
                    bias=neg_mx[:, 0:1],
                    scale=1.0,
                    accum_out=sumexp,
                )
                rsum = sm_pool.tile([1, 1], f32)
                nc.vector.reciprocal(rsum, sumexp)
                alpha = sm_pool.tile([1, CHI], f32)
                nc.vector.tensor_scalar_mul(alpha, exps, rsum)

                nc.tensor.transpose(a_psum, alpha, ident[:1, :1])
                a_one = sm_pool.tile([CHI, 1], f32)
                nc.scalar.copy(out=a_one, in_=a_psum)

                for s in range(NW):
                    nc.tensor.matmul(
                        a_pat[:, s : s + 1], ind1[:, s, :], a_one,
                        start=True, stop=True,
                    )
                rhs = rhs_pool.tile([P, NW, NG], f32)
                rhss[b] = rhs
                for s in range(NW):
                    nc.vector.tensor_scalar_mul(
                        rhs[:, s, :], ind2[:, s, :], a_pat[:, s : s + 1]
                    )

            def emit_stage2(b):
                want = wants[b]
                rhs = rhss[b]
                ob = pso_pool.tile([P, NCHUNK, NG], f32)
                want_w = want.rearrange("p (m s) -> p s m", s=NW)
                for W in range(NCHUNK):
                    if "s2" in ablate:
                        nc.tensor.matmul(ob[:, W, :], wants[b][:, :P], rhs[:, 0, :], start=True, stop=True)
                        continue
                    for s in range(NW):
                        nc.tensor.matmul(
                            ob[:, W, :],
                            want_w[:, s, W * P : (W + 1) * P],
                            rhs[:, s, :],
                            start=(s == 0),
                            stop=(s == NW - 1),
                        )
                out_sb = osb_pool.tile([P, NCHUNK, NG], f32)
                nc.scalar.copy(out=out_sb, in_=ob)
                nc.sync.dma_start(
                    out=y_d[b].rearrange("(W m g) -> m W g", W=NCHUNK, g=NG),
                    in_=out_sb,
                )

            def emit_all():
                for b in range(S + 2):
                    if 0 <= b - 1 < S:
                        emit_stage1_dve(b - 1)
                    if b < S:
                        emit_load_transpose(b)
                    if 0 <= b - 2 < S:
                        emit_stage2(b - 2)
                    if 0 <= b - 1 < S:
                        emit_stage1_rest(b - 1)

            if repeat == 1:
                emit_all()
            elif repeat < 0:
                with tc.For_i(0, -repeat, 1):
                    emit_all()
            else:
                for _rep in range(repeat):
                    emit_all()

    nc.compile()
    return nc


def _scatter_maps():
    s_idx = np.arange(5)[:, None]
    p_idx = np.arange(P)[None, :]
    cmap = (128 * s_idx + p_idx) % CHI  # [5, P]
    gmap = (128 * s_idx + p_idx) // CHI  # [5, P]
    ind1 = (np.arange(CHI)[None, :, None] == cmap[:, None, :]).astype(np.float32)
    ind2 = (np.arange(32)[None, None, :] == gmap[:, :, None]).astype(np.float32)
    return ind1, ind2


def _host_inputs(xs):
    """Global (all-core concatenated) input arrays keyed by dram tensor name.

    xs: [B, CHI*D] float32 (row-major flat samples).
    """
    if KIND == "t":
        # want[b, q, f] = xs[b, 128*f + q], as bf16
        xt = np.ascontiguousarray(
            xs.reshape(B, NB, P).transpose(0, 2, 1).astype(BF16)
        )
        ind1, ind2 = _scatter_maps()
        return {
            "x": xt,
            "ind1": np.tile(ind1, (N_CORES, 1, 1)),
            "ind2": np.tile(ind2.astype(BF16), (N_CORES, 1, 1)),
        }
    feed = {"x": xs}
    if KIND == "y":
        ind1, ind2 = _scatter_maps()
        feed["ident"] = np.tile(np.eye(P, dtype=np.float32), (N_CORES, 1))
        feed["ind1"] = np.tile(ind1, (N_CORES, 1, 1))
        feed["ind2"] = np.tile(ind2, (N_CORES, 1, 1))
    return feed


def _unpermute_y(y_raw):
    """Invert the device output permutation of the "t" kernel.

    Device wrote y_raw[b, (32t+g)*512 + n] = y[b, 16384t + 32n + g].
    """
    if KIND != "t":
        return y_raw
    return np.ascontiguousarray(
        y_raw.reshape(B, 4, 32, 512).transpose(0, 1, 3, 2)
    ).reshape(B, D)


def _get_nc():
    if "nc" not in _CACHE:
        if KIND == "t":
            _CACHE["nc"] = _build_nc_t()
        elif KIND == "y":
            _CACHE["nc"] = _build_nc_y()
        else:
            _CACHE["nc"] = _build_nc()
    return _CACHE["nc"]


def _get_runner():
    if "runner" not in _CACHE:
        run, sharded, mesh, body = _make_runner(_get_nc())
        _CACHE["sharded"] = sharded
        _CACHE["mesh"] = mesh
        _CACHE["body"] = body
        _CACHE["runner"] = run
    return _CACHE["runner"]


def _make_runner(nc):
    """Compile once and return f(x_global[64, CHI*D]) -> y_global[64, D]."""
    import jax
    from jax.sharding import Mesh, PartitionSpec
    from jax.experimental.shard_map import shard_map
    from concourse import bass2jax, mybir

    bass2jax.install_neuronx_cc_hook()

    partition_name = (
        nc.partition_id_tensor.name if nc.partition_id_tensor else None
    )
    in_names = []
    out_names = []
    out_avals = []
    zero_outs = []
    for alloc in nc.m.functions[0].allocations:
        if not isinstance(alloc, mybir.MemoryLocationSet):
            continue
        name = alloc.memorylocations[0].name
        if alloc.kind == "ExternalInput":
            if name != partition_name:
                in_names.append(name)
        elif alloc.kind == "ExternalOutput":
            shape = tuple(alloc.tensor_shape)
            dtype = mybir.dt.np(alloc.dtype)
            out_avals.append(jax.core.ShapedArray(shape, dtype))
            out_names.append(name)
            zero_outs.append(np.zeros(shape, dtype))
    n_params = len(in_names)
    n_outs = len(out_avals)
    in_names.extend(out_names)
    donate = tuple(range(n_params, n_params + n_outs))

    def _body(*args):
        operands = list(args)
        if partition_name is not None:
            operands.append(bass2jax.partition_id_tensor())
            in_full = tuple(in_names) + (partition_name,)
        else:
            in_full = tuple(in_names)
        outs = bass2jax._bass_exec_p.bind(
            *operands,
            out_avals=tuple(out_avals),
            in_names=in_full,
            out_names=tuple(out_names),
            lowering_input_output_aliases=(),
            sim_require_finite=True,
            sim_require_nnan=True,
            nc=nc,
        )
        return tuple(outs)

    devices = jax.devices()[:N_CORES]
    mesh = Mesh(np.asarray(devices), ("core",))
    in_specs = (PartitionSpec("core"),) * (n_params + n_outs)
    out_specs = (PartitionSpec("core"),) * len(out_names)
    sharded = jax.jit(
        shard_map(
            _body, mesh=mesh, in_specs=in_specs, out_specs=out_specs, check_rep=False
        ),
        donate_argnums=donate,
        keep_unused=True,
    )

    param_names = in_names[:n_params]
    _CACHE["param_names"] = param_names

    def run(xs):
        feed = _host_inputs(xs)
        args = [feed[n] for n in param_names]
        concat_zeros = [
            np.zeros((N_CORES * z.shape[0], *z.shape[1:]), z.dtype) for z in zero_outs
        ]
        return sharded(*args, *concat_zeros)[0]

    return run, sharded, mesh, _body


def kernel(**inputs):
    x = np.ascontiguousarray(np.asarray(inputs["x"], dtype=np.float32))
    assert x.shape == (B, CHI, 64, 32, 32), x.shape
    xs = x.reshape(B, CHI * D)
    run = _get_runner()
    last_err = None
    for _attempt in range(3):
        try:
            out = np.asarray(run(xs))
            break
        except Exception as e:  # transient NRT device errors: retry
            last_err = e
    else:
        raise last_err
    out = _unpermute_y(out)
    return out.reshape(B, 64, 32, 32)
